# revision 1
# baseline (speedup 1.0000x reference)
"""CapsuleLayer kernel for 8 Trainium2 NeuronCores.

Math: with b0 = 0, softmax(b0, axis=1) is exactly uniform (1/N), so
outputs[b,i,k] = squash_k((1/N) * sum_j inputs_hat[b,j,k]) independent of i.
The b-update keeps b constant along axis 1, so softmax stays exactly uniform
and all routing iterations return the same outputs. Hence:

    Wsum[m,k] = sum_j W[j,m,k]
    v[b,k]    = (1/N) * (inputs @ Wsum)[b,k]
    out[b,i,k] = squash_k(v)[b,k]          (broadcast over i)

Kernel 1 (m-sharded): core c reduces W[:, 32c:32c+32, :] over j -> Wsum rows.
Kernel 2 (batch-sharded): core c computes squash((inputs_c @ Wsum)/N) and
broadcast-writes its [64, 256, 256] output slice.
"""

import numpy as np

import concourse.bass as bass
import concourse.mybir as mybir
import concourse.tile as tile
from concourse.ap import AP
from concourse.bass_utils import run_bass_kernel_spmd

F32 = mybir.dt.float32

B, N = 512, 256
NCORES = 8
BPC = B // NCORES  # 64 batch rows per core (kernel 2)
MPC = N // NCORES  # 32 m rows per core (kernel 1)
REPS = 64          # output i-rows written per partition per output DMA
EPS = 1e-7

_CACHE = {}


def _fix_multiwait(nc, maxw=1):
    """This walrus build rejects instructions carrying more than one sync
    wait ("Too many sync wait commands"). Hoist extra waits into standalone
    single-wait EventSemaphore instructions on the same engine, placed
    immediately before the offender."""
    ctr = 0
    for fn in nc.m.functions:
        for bb in fn.blocks:
            out = []
            for ins in bb.instructions:
                si = ins.sync_info
                if si is not None and len(si.on_wait) > maxw:
                    waits = list(si.on_wait)
                    for w in waits[:-maxw]:
                        ctr += 1
                        ev = mybir.InstEventSemaphore(
                            name=f"mwsplit-{ctr}",
                            engine=ins.engine,
                            ins=[],
                            outs=[],
                            sync_info=mybir.SyncInfo(on_wait=[w], on_update=[]),
                        )
                        nc.register_instruction(ev, overwrite=True)
                        out.append(ev)
                    si.on_wait = waits[-maxw:]
                    ins.sync_info = si
                out.append(ins)
            bb.instructions[:] = out
    return nc

# Exec times (ns) of the last traced run, for test harnesses.
LAST_EXEC_NS = {"k1": None, "k2": None}


def _build_k1():
    """Reduce the per-core W slice over j.

    Input  w_in [256 (j), 8192 (m_local*256 + k)]  (= W[:, mslice, :] flat)
    Output wsum_part [1, 8192]  (= Wsum[mslice, :] flat)

    Pipeline per chunk: DMA both j-halves, DVE-add them (j 256->128),
    then PE ones-matmuls reduce the 128 partitions; DVE copies PSUM->acc.
    The _fix_multiwait post-pass legalizes any multi-wait instruction, so
    loads/compute overlap freely.
    """
    nc = bass.Bass()
    FREE = MPC * N    # 8192
    MMF = 512         # moving free dim per matmul

    w = nc.dram_tensor("w_in", [N, FREE], F32, kind="ExternalInput")
    wsum = nc.dram_tensor("wsum_part", [1, FREE], F32, kind="ExternalOutput")

    # Chunk sizes: 1 MB loads keep DMA efficiency and let PE start early;
    # small last chunks shorten the serial tail after the final load.
    CHUNKS = [2048, 2048, 2048, 1024, 1024]
    assert sum(CHUNKS) == FREE

    with tile.TileContext(nc) as tc:
        with (
            tc.tile_pool(name="singles", bufs=1) as singles,
            tc.tile_pool(name="psum", bufs=8, space="PSUM") as psum_pool,
        ):
            ones = singles.tile([128, 1], F32)
            nc.vector.memset(ones[:], 1.0)
            acc = singles.tile([1, FREE], F32)

            off = 0
            for ci, chunk in enumerate(CHUNKS):
                sl = slice(off, off + chunk)
                ta = singles.tile([128, chunk], F32, tag=f"ta{ci}")
                nc.sync.dma_start(out=ta[:], in_=w[0:128, sl])
                tb = singles.tile([128, chunk], F32, tag=f"tb{ci}")
                nc.sync.dma_start(out=tb[:], in_=w[128:256, sl])
                ts = singles.tile([128, chunk], F32, tag=f"ts{ci}")
                nc.vector.tensor_add(ts[:], ta[:], tb[:])
                for g in range(chunk // MMF):
                    ps = psum_pool.tile([1, MMF], F32)
                    nc.tensor.matmul(
                        ps[:], lhsT=ones[:], rhs=ts[:, g * MMF:(g + 1) * MMF],
                        start=True, stop=True,
                    )
                    osl = slice(off + g * MMF, off + (g + 1) * MMF)
                    nc.vector.tensor_copy(out=acc[0:1, osl], in_=ps[:])
                off += chunk

            nc.sync.dma_start(out=wsum[:], in_=acc[:])
    return nc


def _build_k2(REPS_=REPS):
    """Per-core: u = inputs_c @ Wsum, s = squash(u/N), broadcast-write output.

    Inputs  xt   [256 (m), 64 (b)]   (= inputs_c.T)
            wsum [256 (m), 256 (k)]
    Output  out  [BPC*N*N] flat = out[b, i, k] with value s[b, k].

    PSUM partition q = 2*b + ihalf (interleaved duplicate of b), so the flat
    output address q*(N*128) + g*(16*N) + t is affine per DMA g.
    """
    nc = bass.Bass()
    xt = nc.dram_tensor("xt", [N, BPC], F32, kind="ExternalInput")
    ws = nc.dram_tensor("wsum", [N, N], F32, kind="ExternalInput")
    out = nc.dram_tensor("out", [BPC * N * N], F32, kind="ExternalOutput")

    SREP_W = REPS_ * N          # output elements per partition per DMA
    NDMA = (N // 2) // REPS_    # output DMAs, one per group of REPS_ i-rows

    with tile.TileContext(nc) as tc:
        with (
            tc.tile_pool(name="sb", bufs=1) as sb,
            tc.tile_pool(name="psum", bufs=1, space="PSUM") as psum_pool,
        ):
            # Load inputs_c.T halves and Wsum halves (contraction dim m on
            # partitions).
            xt0 = sb.tile([128, BPC], F32)
            nc.sync.dma_start(out=xt0[:], in_=xt[0:128, :])
            xt1 = sb.tile([128, BPC], F32)
            nc.sync.dma_start(out=xt1[:], in_=xt[128:256, :])
            # GpSimd (SWDGE) is idle ~1 us before the HWDGE engines clear
            # their preamble; issuing the matmul-gating Wsum loads there
            # starts the serial compute chain earlier.
            ws0 = sb.tile([128, N], F32)
            nc.gpsimd.dma_start(out=ws0[:], in_=ws[0:128, :])
            ws1 = sb.tile([128, N], F32)
            nc.gpsimd.dma_start(out=ws1[:], in_=ws[128:256, :])

            # Duplicate b columns interleaved: xd[:, 2b + d] = xt[:, b].
            # (A stride-0 lhsT AP would avoid the copies, but the BIR
            # verifier requires the stationary operand to have exactly one
            # free dimension.)
            xd0 = sb.tile([128, 2 * BPC], F32)
            xd1 = sb.tile([128, 2 * BPC], F32)
            for xd, xsrc in ((xd0, xt0), (xd1, xt1)):
                pairs = xd[:].rearrange("p (b two) -> p b two", two=2)
                nc.vector.tensor_copy(out=pairs[:, :, 0], in_=xsrc[:])
                nc.vector.tensor_copy(out=pairs[:, :, 1], in_=xsrc[:])

            # u[q, k] = sum_m inputs_c[q//2, m] * Wsum[m, k]
            u = psum_pool.tile([128, N], F32)
            nc.tensor.matmul(u[:], lhsT=xd0[:], rhs=ws0[:], start=True, stop=False)
            nc.tensor.matmul(u[:], lhsT=xd1[:], rhs=ws1[:], start=False, stop=True)

            # squash: v = u/N; s2 = sum_k v^2; s = v * s2/(1+s2)/sqrt(s2+eps)
            #       = u * factor,  factor = s2/(1+s2)/sqrt(s2+eps)/N
            sq = sb.tile([128, N], F32)
            s2 = sb.tile([128, 1], F32)
            nc.scalar.activation(
                out=sq[:], in_=u[:], func=mybir.ActivationFunctionType.Square,
                scale=1.0 / N, accum_out=s2[:],
            )
            eps_t = sb.tile([128, 1], F32)
            nc.vector.memset(eps_t[:], EPS)
            r = sb.tile([128, 1], F32)
            nc.scalar.activation(
                out=r[:], in_=s2[:], func=mybir.ActivationFunctionType.Sqrt,
                bias=eps_t[:],
            )
            den = sb.tile([128, 1], F32)
            nc.vector.scalar_tensor_tensor(
                den[:], s2[:], 1.0, r[:],
                op0=mybir.AluOpType.add, op1=mybir.AluOpType.mult,
            )
            rec = sb.tile([128, 1], F32)
            nc.vector.reciprocal(rec[:], den[:])
            fac = sb.tile([128, 1], F32)
            nc.vector.scalar_tensor_tensor(
                fac[:], s2[:], 1.0 / N, rec[:],
                op0=mybir.AluOpType.mult, op1=mybir.AluOpType.mult,
            )

            # s_row[q, k] = s[q//2, k]
            s_row = sb.tile([128, N], F32)
            nc.vector.tensor_scalar(
                s_row[:], u[:], fac[:], None, mybir.AluOpType.mult
            )

            # DMA g writes out[q*32768 + g*4096 + rep*256 + k] = s_row[q, k]
            # via a stride-0 repeat on the SBUF source:
            # b = q//2, i = (q%2)*128 + g*16 + rep, k.
            src = AP(
                tensor=s_row.tensor,
                offset=s_row[:].offset,
                ap=[s_row[:].ap[0], [0, REPS_], [1, N]],
            )
            for g in range(NDMA):
                dst = AP(
                    tensor=out,
                    offset=g * SREP_W,
                    ap=[[128 * N, 128], [N, REPS_], [1, N]],
                )
                eng = nc.sync if g % 2 == 0 else nc.scalar
                eng.dma_start(out=dst, in_=src)
    return nc


def _run(nc, in_maps, core_ids, trace):
    if trace:
        try:
            return run_bass_kernel_spmd(nc, in_maps, core_ids, trace=True)
        except Exception as e:  # noqa: BLE001
            print(f"kernel: trace run failed ({e}); rerunning without trace")
    return run_bass_kernel_spmd(nc, in_maps, core_ids, trace=False)


def _get(name):
    if name not in _CACHE:
        _CACHE[name] = _fix_multiwait(_build_k1() if name == "k1" else _build_k2())
    return _CACHE[name]


def kernel(inputs: np.ndarray, W: np.ndarray, trace: bool = False) -> np.ndarray:
    inputs = np.ascontiguousarray(inputs, dtype=np.float32)
    W = np.ascontiguousarray(W, dtype=np.float32)
    core_ids = list(range(NCORES))

    # ---- kernel 1: Wsum rows, m-sharded ----
    k1 = _get("k1")
    in_maps1 = [
        {
            "w_in": np.ascontiguousarray(
                W[:, c * MPC:(c + 1) * MPC, :]
            ).reshape(N, MPC * N)
        }
        for c in core_ids
    ]
    res1 = _run(k1, in_maps1, core_ids, trace)
    LAST_EXEC_NS["k1"] = res1.exec_time_ns
    wsum = np.concatenate(
        [res1.results[c]["wsum_part"].reshape(MPC, N) for c in core_ids], axis=0
    )  # [256, 256]

    # ---- kernel 2: squash + broadcast write, batch-sharded ----
    k2 = _get("k2")
    xt_full = np.ascontiguousarray(inputs.T)  # [256, 512]
    in_maps2 = [
        {
            "xt": np.ascontiguousarray(xt_full[:, c * BPC:(c + 1) * BPC]),
            "wsum": wsum,
        }
        for c in core_ids
    ]
    res2 = _run(k2, in_maps2, core_ids, trace)
    LAST_EXEC_NS["k2"] = res2.exec_time_ns
    out = np.concatenate(
        [res2.results[c]["out"].reshape(BPC, N, N) for c in core_ids], axis=0
    )
    return out



# revision 3
# speedup vs baseline: 1.5634x; 1.5634x over previous
"""CapsuleLayer kernel for 8 Trainium2 NeuronCores.

Math: with b0 = 0, softmax(b0, axis=1) is exactly uniform (1/N), so
outputs[b,i,k] = squash_k((1/N) * sum_j inputs_hat[b,j,k]) independent of i.
The b-update keeps b constant along axis 1, so softmax stays exactly uniform
and all routing iterations return the same outputs. Hence:

    Wsum[m,k] = sum_j W[j,m,k]
    v[b,k]    = (1/N) * (inputs @ Wsum)[b,k]
    out[b,i,k] = squash_k(v)[b,k]          (broadcast over i)

Precision: W is fed to the device as bf16 (halves the HBM read) and the
output is written as bf16 (halves the HBM write), with fp32 accumulation
everywhere on-chip. Measured end-to-end rel err ~2.5e-3 vs the fp32
reference (gate is 2e-2).

Kernel 1 (j-sharded): core c reduces W[32c:32c+32] over j via PE matmuls
against a constant block-diagonal eye64 (stationary loaded once), giving a
[64, 1024] fp32 partial of Wsum; host sums the 8 partials.
Kernel 2 (batch-sharded): core c computes squash((inputs_c @ Wsum)/N) and
broadcast-writes its [64, 256, 256] output slice as bf16 using 4 KB
descriptors (8 contiguous row-copies staged in SBUF).
"""

import numpy as np
import ml_dtypes

import concourse.bass as bass
import concourse.mybir as mybir
import concourse.tile as tile
from concourse.ap import AP
from concourse.bass_utils import run_bass_kernel_spmd

F32 = mybir.dt.float32
BF16 = mybir.dt.bfloat16

B, N = 512, 256
NCORES = 8
BPC = B // NCORES   # 64 batch rows per core (kernel 2)
JPC = N // NCORES   # 32 j rows per core (kernel 1)
EPS = 1e-7

# kernel 1 tiling: TJ j-rows per tile -> 128 partitions = TJ*SUB, SUB subs
TJ = 2
SUB = 128 // TJ            # 64 output partitions
NT = JPC // TJ             # 16 tiles
TF = (N * N) // SUB        # 1024 free elements per tile

# kernel 2 output: C contiguous copies per descriptor, 2 output DMAs
C = 8
NDMA = 2
REPS = (N // 2) // NDMA    # 64 i-replicas per partition per DMA

_CACHE = {}


def _fix_multiwait(nc, maxw=1):
    """This walrus build rejects instructions carrying more than one sync
    wait ("Too many sync wait commands"). Hoist extra waits into standalone
    single-wait EventSemaphore instructions on the same engine, placed
    immediately before the offender."""
    ctr = 0
    for fn in nc.m.functions:
        for bb in fn.blocks:
            out = []
            for ins in bb.instructions:
                si = ins.sync_info
                if si is not None and len(si.on_wait) > maxw:
                    waits = list(si.on_wait)
                    for w in waits[:-maxw]:
                        ctr += 1
                        ev = mybir.InstEventSemaphore(
                            name=f"mwsplit-{ctr}",
                            engine=ins.engine,
                            ins=[],
                            outs=[],
                            sync_info=mybir.SyncInfo(on_wait=[w], on_update=[]),
                        )
                        nc.register_instruction(ev, overwrite=True)
                        out.append(ev)
                    si.on_wait = waits[-maxw:]
                    ins.sync_info = si
                out.append(ins)
            bb.instructions[:] = out
    return nc

# Exec times (ns) of the last traced run, for test harnesses.
LAST_EXEC_NS = {"k1": None, "k2": None}


def _build_k1():
    """Partial Wsum over this core's 32 j-rows.

    Input  wj  [NT, 128, TF] bf16  (= W[jslice] with partition p = jl*SUB+sub,
                                      flat mk = sub*TF + f)
    Input  eye [128, SUB]    bf16  (eye[p, q] = p % SUB == q)
    Output part [SUB, TF]    fp32  (part[q, f] = sum_{j in slice} W[j, q*TF+f])

    All NT tile loads stream on both HWDGE queues; the PE accumulates every
    tile into two persistent PSUM banks via matmuls against the constant
    eye stationary, so no DVE adds and only two PSUM->SBUF copies at the end.
    """
    nc = bass.Bass()
    GW = 512                  # fp32 PSUM bank = 512 floats -> 2 f-groups
    NG = TF // GW

    wj = nc.dram_tensor("wj", [NT, 128, TF], BF16, kind="ExternalInput")
    eye = nc.dram_tensor("eye", [128, SUB], BF16, kind="ExternalInput")
    part = nc.dram_tensor("part", [SUB, TF], F32, kind="ExternalOutput")

    with tile.TileContext(nc) as tc:
        with (
            tc.tile_pool(name="sb", bufs=1) as sb,
            tc.tile_pool(name="psum", bufs=1, space="PSUM") as psum_pool,
        ):
            eye_t = sb.tile([128, SUB], BF16)
            nc.gpsimd.dma_start(out=eye_t[:], in_=eye[:, :])

            tiles = []
            for t in range(NT):
                wt = sb.tile([128, TF], BF16, tag=f"w{t}")
                eng = nc.sync if t % 2 == 0 else nc.scalar
                eng.dma_start(out=wt[:], in_=wj[t, :, :])
                tiles.append(wt)

            ps = []
            for g in range(NG):
                psg = psum_pool.tile([SUB, GW], F32, tag=f"ps{g}")
                ps.append(psg)
            for t in range(NT):
                for g in range(NG):
                    nc.tensor.matmul(
                        ps[g][:], lhsT=eye_t[:],
                        rhs=tiles[t][:, g * GW:(g + 1) * GW],
                        start=(t == 0), stop=(t == NT - 1),
                    )

            acc = sb.tile([SUB, TF], F32)
            for g in range(NG):
                nc.vector.tensor_copy(
                    out=acc[:, g * GW:(g + 1) * GW], in_=ps[g][:]
                )
            nc.sync.dma_start(out=part[:, :], in_=acc[:])
    return nc


def _build_k2():
    """Per-core: u = inputs_c @ Wsum, s = squash(u/N), broadcast-write bf16.

    Inputs  xt   [256 (m), 64 (b)] fp32   (= inputs_c.T)
            ws   [2, 128, 256]     fp32   (= Wsum halves, m on partitions)
    Output  out  [BPC*N*N] flat bf16 = out[b, i, k] with value s[b, k].

    PSUM partition q = 2*b + ihalf (interleaved duplicate of b), so the flat
    output address q*(N*128) + ... is affine per DMA. The SBUF source stages
    C=8 contiguous copies of each row so every descriptor is C*N*2 = 4 KB.
    """
    nc = bass.Bass()
    xt = nc.dram_tensor("xt", [N, BPC], F32, kind="ExternalInput")
    ws = nc.dram_tensor("ws", [2, 128, N], F32, kind="ExternalInput")
    out = nc.dram_tensor("out", [BPC * N * N], BF16, kind="ExternalOutput")

    with tile.TileContext(nc) as tc:
        with (
            tc.tile_pool(name="sb", bufs=1) as sb,
            tc.tile_pool(name="psum", bufs=1, space="PSUM") as psum_pool,
        ):
            # GpSimd (SWDGE) is live ~1 us before the HWDGE engines clear
            # their preamble; issuing the matmul-gating Wsum loads there
            # starts the serial compute chain earlier.
            ws0 = sb.tile([128, N], F32)
            nc.gpsimd.dma_start(out=ws0[:], in_=ws[0, :, :])
            ws1 = sb.tile([128, N], F32)
            nc.gpsimd.dma_start(out=ws1[:], in_=ws[1, :, :])
            xt0 = sb.tile([128, BPC], F32)
            nc.sync.dma_start(out=xt0[:], in_=xt[0:128, :])
            xt1 = sb.tile([128, BPC], F32)
            nc.scalar.dma_start(out=xt1[:], in_=xt[128:256, :])

            # Duplicate b columns interleaved: xd[:, 2b + d] = xt[:, b].
            xd0 = sb.tile([128, 2 * BPC], F32)
            xd1 = sb.tile([128, 2 * BPC], F32)
            for xd, xsrc in ((xd0, xt0), (xd1, xt1)):
                pairs = xd[:].rearrange("p (b two) -> p b two", two=2)
                nc.vector.tensor_copy(out=pairs[:, :, 0], in_=xsrc[:])
                nc.vector.tensor_copy(out=pairs[:, :, 1], in_=xsrc[:])

            # u[q, k] = sum_m inputs_c[q//2, m] * Wsum[m, k]
            u = psum_pool.tile([128, N], F32)
            nc.tensor.matmul(u[:], lhsT=xd0[:], rhs=ws0[:], start=True, stop=False)
            nc.tensor.matmul(u[:], lhsT=xd1[:], rhs=ws1[:], start=False, stop=True)

            # squash: v = u/N; s2 = sum_k v^2; s = u * factor,
            #         factor = s2/(1+s2)/sqrt(s2+eps)/N
            sq = sb.tile([128, N], F32)
            s2 = sb.tile([128, 1], F32)
            nc.scalar.activation(
                out=sq[:], in_=u[:], func=mybir.ActivationFunctionType.Square,
                scale=1.0 / N, accum_out=s2[:],
            )
            eps_t = sb.tile([128, 1], F32)
            nc.vector.memset(eps_t[:], EPS)
            r = sb.tile([128, 1], F32)
            nc.scalar.activation(
                out=r[:], in_=s2[:], func=mybir.ActivationFunctionType.Sqrt,
                bias=eps_t[:],
            )
            den = sb.tile([128, 1], F32)
            nc.vector.scalar_tensor_tensor(
                den[:], s2[:], 1.0, r[:],
                op0=mybir.AluOpType.add, op1=mybir.AluOpType.mult,
            )
            rec = sb.tile([128, 1], F32)
            nc.vector.reciprocal(rec[:], den[:])
            fac = sb.tile([128, 1], F32)
            nc.vector.scalar_tensor_tensor(
                fac[:], s2[:], 1.0 / N, rec[:],
                op0=mybir.AluOpType.mult, op1=mybir.AluOpType.mult,
            )

            # s_rep[q, :] = C contiguous bf16 copies of s[q//2, :]
            s_rep = sb.tile([128, C * N], BF16)
            nc.vector.tensor_scalar(
                s_rep[:, 0:N], u[:], fac[:], None, mybir.AluOpType.mult
            )
            w = N
            while w < C * N:
                nc.vector.tensor_copy(out=s_rep[:, w:2 * w], in_=s_rep[:, 0:w])
                w *= 2

            # DMA g writes out[q*32768 + g*REPS*256 + r*256 + k] = s_rep[q, k%N]
            # with C copies per descriptor (inner contiguous run = C*N el).
            src = AP(
                tensor=s_rep.tensor,
                offset=s_rep[:].offset,
                ap=[s_rep[:].ap[0], [0, REPS // C], [1, C * N]],
            )
            for g in range(NDMA):
                dst = AP(
                    tensor=out,
                    offset=g * REPS * N,
                    ap=[[128 * N, 128], [C * N, REPS // C], [1, C * N]],
                )
                eng = nc.sync if g % 2 == 0 else nc.scalar
                eng.dma_start(out=dst, in_=src)
    return nc


def _run(nc, in_maps, core_ids, trace):
    if trace:
        try:
            return run_bass_kernel_spmd(nc, in_maps, core_ids, trace=True)
        except Exception as e:  # noqa: BLE001
            print(f"kernel: trace run failed ({e}); rerunning without trace")
    return run_bass_kernel_spmd(nc, in_maps, core_ids, trace=False)


def _get(name):
    if name not in _CACHE:
        _CACHE[name] = _fix_multiwait(_build_k1() if name == "k1" else _build_k2())
    return _CACHE[name]


def kernel(inputs: np.ndarray, W: np.ndarray, trace: bool = False) -> np.ndarray:
    inputs = np.ascontiguousarray(inputs, dtype=np.float32)
    W = np.ascontiguousarray(W, dtype=np.float32)
    core_ids = list(range(NCORES))

    # ---- kernel 1: partial Wsum, j-sharded, bf16 reads ----
    k1 = _get("k1")
    W_bf = W.astype(ml_dtypes.bfloat16)  # [256, 256, 256] contiguous
    eye = np.zeros((128, SUB), dtype=ml_dtypes.bfloat16)
    eye[np.arange(128), np.arange(128) % SUB] = 1
    in_maps1 = [
        {
            "wj": W_bf[c * JPC:(c + 1) * JPC].reshape(NT, 128, TF),
            "eye": eye,
        }
        for c in core_ids
    ]
    res1 = _run(k1, in_maps1, core_ids, trace)
    LAST_EXEC_NS["k1"] = res1.exec_time_ns
    parts = np.stack([res1.results[c]["part"] for c in core_ids])  # [8, SUB, TF]
    wsum = parts.sum(axis=0, dtype=np.float32).reshape(N, N)

    # ---- kernel 2: squash + broadcast bf16 write, batch-sharded ----
    k2 = _get("k2")
    xt_full = np.ascontiguousarray(inputs.T)  # [256, 512]
    ws_in = np.ascontiguousarray(wsum.reshape(2, 128, N))
    in_maps2 = [
        {
            "xt": np.ascontiguousarray(xt_full[:, c * BPC:(c + 1) * BPC]),
            "ws": ws_in,
        }
        for c in core_ids
    ]
    res2 = _run(k2, in_maps2, core_ids, trace)
    LAST_EXEC_NS["k2"] = res2.exec_time_ns
    out = np.empty((B, N, N), dtype=np.float32)
    for c in core_ids:
        out[c * BPC:(c + 1) * BPC] = (
            res2.results[c]["out"].reshape(BPC, N, N).astype(np.float32)
        )
    return out


# revision 8
# speedup vs baseline: 1.7055x; 1.0909x over previous
"""CapsuleLayer kernel for 8 Trainium2 NeuronCores.

Math: with b0 = 0, softmax(b0, axis=1) is exactly uniform (1/N), so
outputs[b,i,k] = squash_k((1/N) * sum_j inputs_hat[b,j,k]) independent of i.
The b-update keeps b constant along axis 1, so softmax stays exactly uniform
and all routing iterations return the same outputs. Hence:

    Wsum[m,k] = sum_j W[j,m,k]
    v[b,k]    = (1/N) * (inputs @ Wsum)[b,k]
    out[b,i,k] = squash_k(v)[b,k]          (broadcast over i)

Precision: W is fed to the device as bf16 (halves the HBM read) and the
output is written as bf16 (halves the HBM write), with fp32 accumulation
everywhere on-chip. Measured end-to-end rel err ~2.5e-3 vs the fp32
reference (gate is 2e-2).

Kernel 1 (j-sharded): core c reduces W[32c:32c+32] over j via PE matmuls
against a constant block-diagonal eye64 (stationary loaded once), giving a
[64, 1024] fp32 partial of Wsum; host sums the 8 partials.
Kernel 2 (batch-sharded): core c computes squash((inputs_c @ Wsum)/N) and
broadcast-writes its [64, 256, 256] output slice as bf16 using 4 KB
descriptors (8 contiguous row-copies staged in SBUF).
"""

import numpy as np
import ml_dtypes

import concourse.bass as bass
import concourse.mybir as mybir
import concourse.tile as tile
from concourse.ap import AP
from concourse.bass_utils import run_bass_kernel_spmd

F32 = mybir.dt.float32
BF16 = mybir.dt.bfloat16

B, N = 512, 256
NCORES = 8
BPC = B // NCORES   # 64 batch rows per core (kernel 2)
JPC = N // NCORES   # 32 j rows per core (kernel 1)
EPS = 1e-7

# kernel 1 tiling: TJ j-rows per tile -> 128 partitions = TJ*SUB, SUB subs
TJ = 2
SUB = 128 // TJ            # 64 output partitions
NT = JPC // TJ             # 16 tiles
TF = (N * N) // SUB        # 1024 free elements per tile

# kernel 2 output: C contiguous copies per descriptor, 2 output DMAs
C = 8
NDMA = 2
REPS = (N // 2) // NDMA    # 64 i-replicas per partition per DMA

_CACHE = {}


def _fix_multiwait(nc, maxw=1):
    """This walrus build rejects instructions carrying more than one sync
    wait ("Too many sync wait commands"). Hoist extra waits into standalone
    single-wait EventSemaphore instructions on the same engine, placed
    immediately before the offender."""
    ctr = 0
    for fn in nc.m.functions:
        for bb in fn.blocks:
            out = []
            for ins in bb.instructions:
                si = ins.sync_info
                if si is not None and len(si.on_wait) > maxw:
                    waits = list(si.on_wait)
                    for w in waits[:-maxw]:
                        ctr += 1
                        ev = mybir.InstEventSemaphore(
                            name=f"mwsplit-{ctr}",
                            engine=ins.engine,
                            ins=[],
                            outs=[],
                            sync_info=mybir.SyncInfo(on_wait=[w], on_update=[]),
                        )
                        nc.register_instruction(ev, overwrite=True)
                        out.append(ev)
                    si.on_wait = waits[-maxw:]
                    ins.sync_info = si
                out.append(ins)
            bb.instructions[:] = out
    return nc

# Exec times (ns) of the last traced run, for test harnesses.
LAST_EXEC_NS = {"k1": None, "k2": None}


def _build_k1():
    """Partial Wsum over this core's 32 j-rows.

    Input  wj  [128, NT*TF] bf16  (host-pretransposed so SBUF layout == DRAM
                                    layout: wj[p, t*TF+f] = Wslice[jl=p//SUB
                                    + 2t ...]; see kernel() for the exact map)
    Input  eye [128, SUB]    bf16  (eye[p, q] = p % SUB == q)
    Output part [SUB, TF]    fp32  (part[q, f] = sum_{j in slice} W[j, q*TF+f])

    Four 1 MB loads stream on both HWDGE queues (one dma_start each, 8 KB
    descriptors); the PE accumulates every tile into two persistent PSUM
    banks via matmuls against the constant eye stationary, so no DVE adds
    and only two PSUM->SBUF copies at the end.
    """
    nc = bass.Bass()
    GW = 512                  # fp32 PSUM bank = 512 floats -> 2 f-groups
    NG = TF // GW
    NLOAD = 4
    TPL = NT // NLOAD         # tiles per load
    LW = TPL * TF             # free width per load (4096)

    wj = nc.dram_tensor("wj", [128, NT * TF], BF16, kind="ExternalInput")
    eye = nc.dram_tensor("eye", [128, SUB], BF16, kind="ExternalInput")
    part = nc.dram_tensor("part", [SUB, TF], F32, kind="ExternalOutput")

    with tile.TileContext(nc) as tc:
        with (
            tc.tile_pool(name="sb", bufs=1) as sb,
            tc.tile_pool(name="psum", bufs=1, space="PSUM") as psum_pool,
        ):
            eye_t = sb.tile([128, SUB], BF16)
            nc.gpsimd.dma_start(out=eye_t[:], in_=eye[:, :])

            tiles = []
            for d in range(NLOAD):
                wt = sb.tile([128, LW], BF16, tag=f"w{d}")
                eng = nc.sync if d % 2 == 0 else nc.scalar
                eng.dma_start(out=wt[:], in_=wj[:, d * LW:(d + 1) * LW])
                tiles.append(wt)

            ps = []
            for g in range(NG):
                psg = psum_pool.tile([SUB, GW], F32, tag=f"ps{g}")
                ps.append(psg)
            for t in range(NT):
                for g in range(NG):
                    nc.tensor.matmul(
                        ps[g][:], lhsT=eye_t[:],
                        rhs=tiles[t // TPL][
                            :, (t % TPL) * TF + g * GW:
                               (t % TPL) * TF + (g + 1) * GW],
                        start=(t == 0), stop=(t == NT - 1),
                    )

            acc = sb.tile([SUB, TF], F32)
            nc.vector.tensor_copy(out=acc[:, 0:GW], in_=ps[0][:])
            nc.scalar.activation(
                out=acc[:, GW:2 * GW], in_=ps[1][:],
                func=mybir.ActivationFunctionType.Copy,
            )
            nc.sync.dma_start(out=part[:, :], in_=acc[:])
    return nc


def _build_k2():
    """Per-core: u = inputs_c @ Wsum, s = squash(u/N), broadcast-write bf16.

    Inputs  xt   [256 (m), 64 (b)] fp32   (= inputs_c.T)
            ws   [2, 128, 256]     fp32   (= Wsum halves, m on partitions)
    Output  out  [BPC*N*N] flat bf16 = out[b, i, k] with value s[b, k].

    PSUM partition q = 2*b + ihalf (interleaved duplicate of b), so the flat
    output address q*(N*128) + ... is affine per DMA. The SBUF source stages
    C=8 contiguous copies of each row so every descriptor is C*N*2 = 4 KB.
    """
    nc = bass.Bass()
    xt = nc.dram_tensor("xt", [N, BPC], BF16, kind="ExternalInput")
    ws = nc.dram_tensor("ws", [2, 128, N], BF16, kind="ExternalInput")
    out = nc.dram_tensor("out", [BPC * N * N], BF16, kind="ExternalOutput")

    with tile.TileContext(nc) as tc:
        with (
            tc.tile_pool(name="sb", bufs=1) as sb,
            tc.tile_pool(name="psum", bufs=1, space="PSUM") as psum_pool,
        ):
            # All four loads are tiny (16-64 KB); interleave them across the
            # two HWDGE queues so the whole serial chain starts ASAP.
            xt0 = sb.tile([128, BPC], BF16)
            nc.sync.dma_start(out=xt0[:], in_=xt[0:128, :])
            ws0 = sb.tile([128, N], BF16)
            nc.scalar.dma_start(out=ws0[:], in_=ws[0, :, :])
            ws1 = sb.tile([128, N], BF16)
            nc.sync.dma_start(out=ws1[:], in_=ws[1, :, :])
            xt1 = sb.tile([128, BPC], BF16)
            nc.scalar.dma_start(out=xt1[:], in_=xt[128:256, :])

            # Duplicate b columns interleaved: xd[:, 2b + d] = xt[:, b].
            xd0 = sb.tile([128, 2 * BPC], BF16)
            xd1 = sb.tile([128, 2 * BPC], BF16)
            for xd, xsrc in ((xd0, xt0), (xd1, xt1)):
                pairs = xd[:].rearrange("p (b two) -> p b two", two=2)
                nc.vector.tensor_copy(out=pairs[:, :, 0], in_=xsrc[:])
                nc.vector.tensor_copy(out=pairs[:, :, 1], in_=xsrc[:])

            # u[q, k] = sum_m inputs_c[q//2, m] * Wsum[m, k]
            u = psum_pool.tile([128, N], F32)
            nc.tensor.matmul(u[:], lhsT=xd0[:], rhs=ws0[:], start=True, stop=False)
            nc.tensor.matmul(u[:], lhsT=xd1[:], rhs=ws1[:], start=False, stop=True)

            # squash: v = u/N; s2 = sum_k v^2; s = u * factor,
            #         factor = s2/(1+s2)/sqrt(s2+eps)/N
            sq = sb.tile([128, N], F32)
            s2 = sb.tile([128, 1], F32)
            nc.scalar.activation(
                out=sq[:], in_=u[:], func=mybir.ActivationFunctionType.Square,
                scale=1.0 / N, accum_out=s2[:],
            )
            eps_t = sb.tile([128, 1], F32)
            nc.vector.memset(eps_t[:], EPS)
            r = sb.tile([128, 1], F32)
            nc.scalar.activation(
                out=r[:], in_=s2[:], func=mybir.ActivationFunctionType.Sqrt,
                bias=eps_t[:],
            )
            den = sb.tile([128, 1], F32)
            nc.vector.scalar_tensor_tensor(
                den[:], s2[:], 1.0, r[:],
                op0=mybir.AluOpType.add, op1=mybir.AluOpType.mult,
            )
            rec = sb.tile([128, 1], F32)
            nc.vector.reciprocal(rec[:], den[:])
            fac = sb.tile([128, 1], F32)
            nc.vector.scalar_tensor_tensor(
                fac[:], s2[:], 1.0 / N, rec[:],
                op0=mybir.AluOpType.mult, op1=mybir.AluOpType.mult,
            )

            # s_rep[q, :] = C contiguous bf16 copies of s[q//2, :]
            s_rep = sb.tile([128, C * N], BF16)
            nc.vector.tensor_scalar(
                s_rep[:, 0:N], u[:], fac[:], None, mybir.AluOpType.mult
            )
            w = N
            while w < C * N:
                nc.vector.tensor_copy(out=s_rep[:, w:2 * w], in_=s_rep[:, 0:w])
                w *= 2

            # DMA g writes out[q*32768 + g*REPS*256 + r*256 + k] = s_rep[q, k%N]
            # with C copies per descriptor (inner contiguous run = C*N el).
            src = AP(
                tensor=s_rep.tensor,
                offset=s_rep[:].offset,
                ap=[s_rep[:].ap[0], [0, REPS // C], [1, C * N]],
            )
            for g in range(NDMA):
                dst = AP(
                    tensor=out,
                    offset=g * REPS * N,
                    ap=[[128 * N, 128], [C * N, REPS // C], [1, C * N]],
                )
                eng = nc.sync if g % 2 == 0 else nc.scalar
                eng.dma_start(out=dst, in_=src)
    return nc


def _run(nc, in_maps, core_ids, trace):
    if trace:
        try:
            return run_bass_kernel_spmd(nc, in_maps, core_ids, trace=True)
        except Exception as e:  # noqa: BLE001
            print(f"kernel: trace run failed ({e}); rerunning without trace")
    return run_bass_kernel_spmd(nc, in_maps, core_ids, trace=False)


def _get(name):
    if name not in _CACHE:
        _CACHE[name] = _fix_multiwait(_build_k1() if name == "k1" else _build_k2())
    return _CACHE[name]


def kernel(inputs: np.ndarray, W: np.ndarray, trace: bool = False) -> np.ndarray:
    inputs = np.ascontiguousarray(inputs, dtype=np.float32)
    W = np.ascontiguousarray(W, dtype=np.float32)
    core_ids = list(range(NCORES))

    # ---- kernel 1: partial Wsum, j-sharded, bf16 reads ----
    # wj[p, t*TF+f] = W[c*JPC + t*TJ + p//SUB, mk=(p%SUB)*TF + f] so the SBUF
    # tile layout equals the DRAM layout (pure-slice 1 MB DMAs).
    k1 = _get("k1")
    W_bf = W.astype(ml_dtypes.bfloat16)  # [256, 256, 256] contiguous
    eye = np.zeros((128, SUB), dtype=ml_dtypes.bfloat16)
    eye[np.arange(128), np.arange(128) % SUB] = 1
    in_maps1 = []
    for c in core_ids:
        a = W_bf[c * JPC:(c + 1) * JPC].reshape(NT, TJ, SUB, TF)
        wj = np.ascontiguousarray(
            a.transpose(1, 2, 0, 3).reshape(128, NT * TF)
        )
        in_maps1.append({"wj": wj, "eye": eye})
    res1 = _run(k1, in_maps1, core_ids, trace)
    LAST_EXEC_NS["k1"] = res1.exec_time_ns
    parts = np.stack([res1.results[c]["part"] for c in core_ids])  # [8, SUB, TF]
    wsum = parts.sum(axis=0, dtype=np.float32).reshape(N, N)

    # ---- kernel 2: squash + broadcast bf16 write, batch-sharded ----
    k2 = _get("k2")
    xt_full = np.ascontiguousarray(inputs.T).astype(ml_dtypes.bfloat16)
    ws_in = np.ascontiguousarray(
        wsum.reshape(2, 128, N).astype(ml_dtypes.bfloat16)
    )
    in_maps2 = [
        {
            "xt": np.ascontiguousarray(xt_full[:, c * BPC:(c + 1) * BPC]),
            "ws": ws_in,
        }
        for c in core_ids
    ]
    res2 = _run(k2, in_maps2, core_ids, trace)
    LAST_EXEC_NS["k2"] = res2.exec_time_ns
    out = np.empty((B, N, N), dtype=np.float32)
    for c in core_ids:
        out[c * BPC:(c + 1) * BPC] = (
            res2.results[c]["out"].reshape(BPC, N, N).astype(np.float32)
        )
    return out
